# revision 9
# baseline (speedup 1.0000x reference)
"""CIN forward kernel for Trainium2, 8 cores.

Symmetric-fold + mixed-dtype-tile pipeline:

Reference computation (per batch b, per position d; n = (b, d) column):
  h0 = x                                  # [F=64, D=64] fields
  for layer l in (1, 2):
    z[(i,j), n] = x[i, n] * h[j, n]       # outer product, K = F*F = 4096
    h[o, n] = relu(sum_c W_l[o, c] z[c, n] + b_l[o])
  y[b] = sum_d concat(h1, h2)[:, b, :] @ wfc + bfc

Distribution: pure data parallel - batch (1024) split 128/core over 8 cores;
weights replicated; no collectives.

Key ideas over the plain 64-chunk pipeline:
  - Layer 1's z1 = x (x) x is SYMMETRIC in (i,j): only 2080 of 4096 products
    are distinct.  We pack the folded products into 17 of the SAME 32 gather
    tiles layer 2 streams (an orientation of K64 with in-degree <= 34 per
    residue assigns every unordered pair {a,b} to a slot (c<17, p) with
    tile_c[p] = x_a and p%64 = b), with folded weights W1f = W1[ab] + W1[ba].
    PE passes drop 64 -> 49 and z-build chunk-ops drop 64 -> 49, with ZERO
    extra DMA (layer 1 free-rides on layer 2's stream).
  - The 10 tiles consumed only by the Pool engine (gpsimd) are streamed in
    fp8-e4m3 (Pool's tensor_tensor cost is dtype-independent; only the x-side
    is quantized, z and W stay fp16 -> final rel err ~9e-3 vs the 2e-2 gate).
    DMA drops 69MB -> ~60MB per core.
  - Tapered 256-col startup/drain blocks; fp8 tiles for 256-blocks are loaded
    512-wide once per block-pair to keep >=512B DMA runs (full DMA rate).
  - W1f stored with duplicated M=128 columns: the idle PE half produces the
    h1 partition-duplicate for free (no cross-partition DMA in the handoff).
  - PE order L1(k+1), L2(k); L2's accumulation runs the 10 Pool-built chunks
    (ready since last iteration) first, then the DVE pieces as they land.
  - d-pooling as ACT accumulate-copies into a persistent pooled tile; the FC
    collapses to ONE K=128 matmul at the end.  Last block's epilogue runs on
    DVE (idle at drain).
"""

import numpy as np

import concourse.bacc as bacc
import concourse.mybir as mybir
import concourse.tile as tile
from concourse.bass_utils import run_bass_kernel_spmd

F = 64          # fields
D = 64          # embedding dim
B = 1024        # full batch
NCORES = 8
B_LOC = B // NCORES          # 128 batches per core
N_TOTAL = B_LOC * D          # 8192 columns per core
MACRO = 512                  # DRAM blocking width
NCH = 32                     # gather tiles (layer-2 chunks)
L1CH = 17                    # folded layer-1 chunks (tiles 0..16)
NT16 = 22                    # fp16 tiles 0..21 (DVE-consumed)
NT8 = 10                     # fp8 tiles 22..31 (Pool-consumed, layer 2 only)
f16 = mybir.dt.float16
f8 = mybir.dt.float8e4
f32 = mybir.dt.float32

# column blocks (n0, width): tapered start/drain; all 256-blocks are
# 512-aligned in pairs so fp8 tile loads can be 512 wide.
BLOCKS = (
    [(i * 256, 256) for i in range(8)]
    + [(2048 + i * 512, 512) for i in range(9)]
    + [(6656 + i * 256, 256) for i in range(6)]
)
NB = len(BLOCKS)
assert sum(w for _, w in BLOCKS) == N_TOTAL

# DVE piece structure (chunk index ranges into the tile list, aligned with
# the A (0..8) / B (9..16) / C (17..21) DMA pieces)
Z2_DVE_PIECES = [(0, 9), (9, 8), (17, 5)]      # tiles 0..21
Z1_PIECES = [(0, 9), (9, 8)]                   # tiles 0..16
Z2_POOL_PIECES = [(22, 5), (27, 5)]            # tiles 22..31


def build_cin_nc():
    nc = bacc.Bacc(None)

    xt = nc.dram_tensor("xt16", [F, N_TOTAL], f16, kind="ExternalInput")
    xg16 = nc.dram_tensor(
        "xg16", [(N_TOTAL // MACRO) * 128, NT16 * MACRO], f16,
        kind="ExternalInput",
    )
    xg8 = nc.dram_tensor(
        "xg8", [(N_TOTAL // MACRO) * 128, NT8 * MACRO], f8,
        kind="ExternalInput",
    )
    w1d = nc.dram_tensor("w1fsb", [128, L1CH * 128], f16, kind="ExternalInput")
    w2d = nc.dram_tensor("w2sb", [128, NCH * F], f16, kind="ExternalInput")
    b1d = nc.dram_tensor("b1dup", [128, 1], f32, kind="ExternalInput")
    b2d = nc.dram_tensor("b2dup", [128, 1], f32, kind="ExternalInput")
    wfcd = nc.dram_tensor("wfccat", [128, 1], f32, kind="ExternalInput")
    bfcd = nc.dram_tensor("bfc", [1, 1], f32, kind="ExternalInput")
    yd = nc.dram_tensor("y", [1, B_LOC], f32, kind="ExternalOutput")

    mult = mybir.AluOpType.mult
    Relu = mybir.ActivationFunctionType.Relu
    Copy = mybir.ActivationFunctionType.Copy

    with tile.TileContext(nc) as tc:
        with (
            tc.tile_pool(name="const", bufs=1) as cpool,
            tc.tile_pool(name="xA", bufs=3) as xapool,
            tc.tile_pool(name="xB", bufs=3) as xbpool,
            tc.tile_pool(name="xC", bufs=2) as xcpool,
            tc.tile_pool(name="x8", bufs=3) as x8pool,
            tc.tile_pool(name="xd", bufs=3) as xdpool,
            tc.tile_pool(name="z1", bufs=2) as z1pool,
            tc.tile_pool(name="z2", bufs=3) as z2pool,
            tc.tile_pool(name="zp", bufs=2) as zppool,
            tc.tile_pool(name="h", bufs=3) as hpool,
            tc.tile_pool(name="psum", bufs=4, space="PSUM") as ppool,
            tc.tile_pool(name="psumfc", bufs=1, space="PSUM") as fcpool,
        ):
            w1 = cpool.tile([128, L1CH * 128], f16)
            w2 = cpool.tile([128, NCH * F], f16)
            b1 = cpool.tile([128, 1], f32)
            b2 = cpool.tile([128, 1], f32)
            wfc = cpool.tile([128, 1], f32)
            bfc = cpool.tile([1, 1], f32)
            pooled = cpool.tile([128, B_LOC], f32)   # [0:64]=L1, [64:128]=L2
            y_sb = cpool.tile([1, B_LOC], f32)
            scr = cpool.tile([128, D], f16)          # discard target for accums

            X16 = {}    # k -> dict piece -> tile ('A' tiles 0..8, 'B' 9..16,
                        #                          'C' 17..21)
            X8s = {}    # k -> (tile, col offset); fp8 tiles 22..31, always
                        # loaded as the full 512-wide macro slice
            xds = {}    # k -> x dup tile
            Hps = {}    # k -> h1 dup tile [128, w] fp16
            h2s = {}    # k -> h2 tile (valid on [64:128])
            z1s = {}    # k -> dict c0 -> (tile, c0, ncch)  folded layer-1 z
            z2s = {}    # k -> dict c0 -> (tile, c0, ncch, col_off)
            ps1 = {}
            ps2 = {}

            X_PIECES = {"A": (0, 9), "B": (9, 8), "C": (17, 5)}
            X_POOLS = {"A": xapool, "B": xbpool, "C": xcpool}

            def load_x16(k, piece):
                n0, w = BLOCKS[k]
                c0, ncch = X_PIECES[piece]
                t = X_POOLS[piece].tile([128, ncch * MACRO], f16, tag="X" + piece)
                rb = (n0 // MACRO) * 128
                sub = n0 % MACRO
                src = xg16[rb : rb + 128, :].rearrange(
                    "p (c n) -> p c n", n=MACRO
                )[:, c0 : c0 + ncch, sub : sub + w]
                nc.sync.dma_start(
                    out=t[:, 0 : ncch * w].rearrange("p (c n) -> p c n", n=w),
                    in_=src,
                )
                X16.setdefault(k, {})[piece] = t

            def load_x8(k):
                """fp8 tiles 22..31 for block k.  Always loads the full
                512-wide macro slice (keeps >=512B DMA runs); a 256-block at
                an odd half reuses its even sibling's tile."""
                n0, w = BLOCKS[k]
                if w == 256 and n0 % MACRO == 256:
                    t, _ = X8s[k - 1]
                    X8s[k] = (t, 256)
                    return
                t = x8pool.tile([128, NT8 * MACRO], f8, tag="X8")
                rb = (n0 // MACRO) * 128
                src = xg8[rb : rb + 128, :].rearrange(
                    "p (c n) -> p c n", n=MACRO
                )
                nc.sync.dma_start(
                    out=t[:].rearrange("p (c n) -> p c n", n=MACRO),
                    in_=src,
                )
                X8s[k] = (t, 0)

            def load_xd(k):
                n0, w = BLOCKS[k]
                xd = xdpool.tile([128, MACRO], f16, tag="xd")
                nc.scalar.dma_start(out=xd[0:64, 0:w], in_=xt[:, n0 : n0 + w])
                nc.scalar.dma_start(out=xd[64:128, 0:w], in_=xt[:, n0 : n0 + w])
                xds[k] = xd

            def z1_piece(k, idx):
                """Folded layer-1 z chunks (DVE), piece idx of Z1_PIECES."""
                n0, w = BLOCKS[k]
                c0, ncch = Z1_PIECES[idx]
                piece, pc0 = ("A", 0) if c0 == 0 else ("B", 9)
                Xt = X16[k][piece]
                # chunk c lives at piece-local index c - pc0
                src = Xt[:, (c0 - pc0) * w : (c0 - pc0 + ncch) * w]
                zfull = z1pool.tile([128, ncch * MACRO], f16, tag=f"z1{idx}")
                z = zfull[:, 0 : ncch * w]
                nc.vector.tensor_tensor(
                    z.rearrange("p (f n) -> p f n", n=w),
                    xds[k][:, 0:w].unsqueeze(1).broadcast_to([128, ncch, w]),
                    src.rearrange("p (f n) -> p f n", n=w),
                    mult,
                )
                z1s.setdefault(k, {})[c0] = (zfull, c0, ncch)

            def z2_dve_piece(k, idx, split=False):
                n0, w = BLOCKS[k]
                c0, ncch = Z2_DVE_PIECES[idx]
                hdup = Hps[k][:, 0:w]

                def emit(c0_, ncch_):
                    if c0_ < 9:
                        piece, pc0 = "A", 0
                    elif c0_ < 17:
                        piece, pc0 = "B", 9
                    else:
                        piece, pc0 = "C", 17
                    Xt = X16[k][piece]
                    src = Xt[:, (c0_ - pc0) * w : (c0_ - pc0 + ncch_) * w]
                    zfull = z2pool.tile(
                        [128, 9 * MACRO], f16, tag="z2"
                    )
                    z = zfull[:, 0 : ncch_ * w]
                    nc.vector.tensor_tensor(
                        z.rearrange("p (f n) -> p f n", n=w),
                        hdup.unsqueeze(1).broadcast_to([128, ncch_, w]),
                        src.rearrange("p (f n) -> p f n", n=w),
                        mult,
                    )
                    z2s.setdefault(k, {})[c0_] = (zfull, c0_, ncch_)

                if not split:
                    emit(c0, ncch)
                else:
                    h1_ = ncch // 2
                    emit(c0, h1_)
                    emit(c0 + h1_, ncch - h1_)

            def z2_pool_piece(k, idx):
                """Layer-2 z chunks on gpsimd from fp8 tiles 22..31."""
                n0, w = BLOCKS[k]
                c0, ncch = Z2_POOL_PIECES[idx]
                hdup = Hps[k][:, 0:w]
                t, coff = X8s[k]
                src = t[:].rearrange(
                    "p (c n) -> p c n", n=MACRO
                )[:, c0 - 22 : c0 - 22 + ncch, coff : coff + w]
                zfull = zppool.tile([128, 5 * MACRO], f16, tag=f"zp{idx}")
                z = zfull[:, 0 : ncch * w]
                nc.gpsimd.tensor_tensor(
                    z.rearrange("p (f n) -> p f n", n=w),
                    hdup.unsqueeze(1).broadcast_to([128, ncch, w]),
                    src,
                    mult,
                )
                z2s.setdefault(k, {})[c0] = (zfull, c0, ncch)

            def pe_layer1(k):
                n0, w = BLOCKS[k]
                ps = ppool.tile([128, MACRO], f32, tag="ps")
                zl = z1s[k]
                for c in range(L1CH):
                    for c0 in zl:
                        zt, zc0, znc = zl[c0]
                        if zc0 <= c < zc0 + znc:
                            off = (c - zc0) * w
                            break
                    nc.tensor.matmul(
                        ps[:, 0:w], w1[:, c * 128 : (c + 1) * 128],
                        zt[:, off : off + w],
                        start=(c == 0), stop=(c == L1CH - 1),
                    )
                ps1[k] = ps
                del z1s[k]

            def pe_layer2(k):
                n0, w = BLOCKS[k]
                ps = ppool.tile([128, MACRO], f32, tag="ps")
                zl = z2s[k]
                # Pool-built chunks (22..31) first: ready since last iteration
                order = list(range(22, 32)) + list(range(0, 22))
                for ci, c in enumerate(order):
                    for c0 in zl:
                        zt, zc0, znc = zl[c0]
                        if zc0 <= c < zc0 + znc:
                            off = (c - zc0) * w
                            break
                    nc.tensor.matmul(
                        ps[64:128, 0:w], w2[:, c * F : (c + 1) * F],
                        zt[:, off : off + w],
                        start=(ci == 0), stop=(ci == NCH - 1),
                    )
                ps2[k] = ps
                del z2s[k]

            def epi1(k):
                n0, w = BLOCKS[k]
                Hp = hpool.tile([128, MACRO], f16, tag="Hp")
                nc.scalar.activation(
                    Hp[:, 0:w], ps1[k][:, 0:w], Relu, bias=b1[:, :]
                )
                Hps[k] = Hp
                del ps1[k]

            def epi2(k):
                n0, w = BLOCKS[k]
                h2 = hpool.tile([128, MACRO], f16, tag="h2")
                if k == NB - 1:
                    # tail: bias+relu on DVE (idle) to skip the ACT hop
                    nc.vector.tensor_scalar(
                        h2[64:128, 0:w], ps2[k][64:128, 0:w], b2[64:128, :],
                        0.0, mybir.AluOpType.add, mybir.AluOpType.max,
                    )
                else:
                    nc.scalar.activation(
                        h2[64:128, 0:w], ps2[k][64:128, 0:w], Relu,
                        bias=b2[64:128, :],
                    )
                h2s[k] = h2
                del ps2[k]

            def red1(k):
                n0, w = BLOCKS[k]
                nb = w // D
                cb = n0 // D
                for j in range(nb):
                    nc.scalar.activation(
                        scr[0:64, :], Hps[k][0:64, j * D : (j + 1) * D], Copy,
                        accum_out=pooled[0:64, cb + j : cb + j + 1],
                    )

            def red2(k):
                n0, w = BLOCKS[k]
                nb = w // D
                cb = n0 // D
                if k == NB - 1:
                    # tail: ACT accums would delay the final FC; DVE is idle
                    nc.vector.tensor_reduce(
                        pooled[64:128, cb : cb + nb],
                        h2s[k][64:128, 0:w].rearrange("p (b d) -> p b d", d=D),
                        mybir.AxisListType.X, mybir.AluOpType.add,
                    )
                else:
                    for j in range(nb):
                        nc.scalar.activation(
                            scr[64:128, :],
                            h2s[k][64:128, j * D : (j + 1) * D], Copy,
                            accum_out=pooled[64:128, cb + j : cb + j + 1],
                        )
                del h2s[k]

            def load_block(k):
                load_xd(k)
                load_x16(k, "A")
                load_x8(k)
                load_x16(k, "B")

            # ---------------- prologue ----------------
            # blocks 0,1: tiles + z1 built immediately; weights interleaved
            load_block(0)
            load_x16(0, "C")
            nc.scalar.dma_start(out=w1[:], in_=w1d[:])
            z1_piece(0, 0)
            z1_piece(0, 1)
            load_block(1)
            nc.scalar.dma_start(out=b1[:], in_=b1d[:])
            nc.scalar.dma_start(out=w2[:], in_=w2d[:])
            nc.scalar.dma_start(out=b2[:], in_=b2d[:])
            nc.scalar.dma_start(out=wfc[:], in_=wfcd[:])
            nc.scalar.dma_start(out=bfc[:], in_=bfcd[:])
            z1_piece(1, 0)
            z1_piece(1, 1)
            pe_layer1(0)
            epi1(0)
            # Pool starts the very first l2 chunks for block 0
            z2_pool_piece(0, 0)
            z2_pool_piece(0, 1)

            # ---------------- steady state ----------------
            # iteration k: PE runs L1(k+1), L2(k).
            #   DVE: z2(k) x3, then z1(k+2) x2 (tiles for k+2 land mid-iter).
            #   Pool: z2_pool(k+1) after epi1(k+1).
            #   DMA: block k+2 piece A/x8/B/xd, then C(k+1).
            for k in range(NB):
                if k + 1 < NB:
                    pe_layer1(k + 1)
                    epi1(k + 1)
                    z2_pool_piece(k + 1, 0)
                z2_dve_piece(k, 0)
                if k + 2 < NB:
                    load_block(k + 2)
                if k + 1 < NB:
                    z2_pool_piece(k + 1, 1)
                z2_dve_piece(k, 1)
                if k + 1 < NB:
                    load_x16(k + 1, "C")
                z2_dve_piece(k, 2, split=(k == NB - 1))
                if k + 2 < NB:
                    z1_piece(k + 2, 0)
                    z1_piece(k + 2, 1)
                pe_layer2(k)
                epi2(k)
                if k + 1 < NB:
                    red1(k + 1)
                if k == 0:
                    red1(0)
                red2(k)

            # ---------------- FC ----------------
            yp = fcpool.tile([1, B_LOC], f32)
            nc.tensor.matmul(yp[:], wfc[:], pooled[:], start=True, stop=True)
            nc.vector.tensor_scalar_add(y_sb[:], yp[:], bfc[:])
            nc.sync.dma_start(out=yd[:], in_=y_sb[:])

    return nc


def _build_fold():
    """Orientation of K64 (+self-loops) assigning every unordered pair {a,b}
    to a slot (c < 17, p) with in1 = x[f[c,p]] and in0 = x[p % 64], while the
    full 32-tile family covers every ORDERED pair exactly once for layer 2.

    Returns f [32, 128] int (the gather patterns)."""
    L1 = [[] for _ in range(F)]
    for a in range(F):
        L1[a].append(a)                      # self loop (a,a) -> residue a
    for a in range(F):
        for b in range(a + 1, F):
            d = (b - a) % F
            if d == 32:
                r, v = a, b                  # tie: to the smaller endpoint
            elif 1 <= d <= 31:
                r, v = b, a
            else:
                r, v = a, b
            L1[r].append(v)
    seqs = np.zeros((F, F), dtype=np.int64)
    for j in range(F):
        s = list(L1[j])
        assert len(s) <= 2 * L1CH
        used = set(s)
        pads = [v for v in range(F) if v not in used]
        while len(s) < 2 * L1CH:
            s.append(pads.pop())
        rest = [v for v in range(F) if v not in set(s)]
        seqs[j] = s + rest
        assert sorted(seqs[j].tolist()) == list(range(F))
    f = np.zeros((NCH, 128), dtype=np.int64)
    for c in range(NCH):
        f[c, 0:64] = seqs[:, 2 * c]
        f[c, 64:128] = seqs[:, 2 * c + 1]
    return f


_FOLD = _build_fold()


def _prep_shared(W1, b1, W2, b2, Wfc, bfc):
    """Host-side weight relayout (replicated on every core)."""
    f = _FOLD

    # layer-1 folded weights, M = 128 duplicated columns
    Wf = np.zeros((F, L1CH, 128), dtype=np.float64)
    assigned = set()
    for c in range(L1CH):
        for p in range(128):
            a = int(f[c, p])
            b = p % 64
            key = (min(a, b), max(a, b))
            if key in assigned:
                continue
            assigned.add(key)
            if a == b:
                Wf[:, c, p] = W1[:, a * F + a]
            else:
                Wf[:, c, p] = W1[:, a * F + b] + W1[:, b * F + a]
    assert len(assigned) == F * (F + 1) // 2
    # w1fsb[p, c*128 + m] = Wf[m % 64, c, p]
    w1fsb = np.ascontiguousarray(
        np.concatenate([Wf, Wf], axis=0)          # [128 m, 17, 128 p]
        .transpose(2, 1, 0).reshape(128, L1CH * 128)
    ).astype(np.float16)

    # layer-2 permuted weights: w2sb[p, c*64 + o] = W2[o, f[c,p]*64 + p%64]
    cols = f * F + (np.arange(128) % 64)[None, :]     # [32, 128]
    w2p = W2[:, cols.reshape(-1)].reshape(F, NCH, 128)  # [o, c, p]
    w2sb = np.ascontiguousarray(
        w2p.transpose(2, 1, 0).reshape(128, NCH * F)
    ).astype(np.float16)

    return {
        "w1fsb": w1fsb,
        "w2sb": w2sb,
        "b1dup": np.concatenate([b1, b1]).reshape(128, 1).astype(np.float32),
        "b2dup": np.concatenate([b2, b2]).reshape(128, 1).astype(np.float32),
        "wfccat": Wfc.reshape(128, 1).astype(np.float32),
        "bfc": bfc.reshape(1, 1).astype(np.float32),
    }


def _prep_x(xtc):
    """Per-macro-blocked gather tile layouts.
    xg16[m*128+p, k*MACRO+nn] = xtc[f[k,p],  m*MACRO+nn]  (tiles 0..21, fp16)
    xg8 [m*128+p, k*MACRO+nn] = xtc[f[22+k,p], m*MACRO+nn] (tiles 22..31, fp8)
    """
    import ml_dtypes

    f = _FOLD
    nm = N_TOTAL // MACRO
    gath = xtc[f.reshape(-1)].reshape(NCH, 128, nm, MACRO)   # [c, p, m, nn]
    g = np.ascontiguousarray(gath.transpose(2, 1, 0, 3))     # [m, p, c, nn]
    xg16 = np.ascontiguousarray(
        g[:, :, 0:NT16, :].reshape(nm * 128, NT16 * MACRO)
    )
    xg8 = np.ascontiguousarray(
        g[:, :, NT16:NCH, :].astype(np.float32)
        .reshape(nm * 128, NT8 * MACRO)
    ).astype(ml_dtypes.float8_e4m3)
    return xg16, xg8


_NC_CACHE = {}


def _get_nc():
    if "nc" not in _NC_CACHE:
        nc = build_cin_nc()
        nc.finalize()
        _NC_CACHE["nc"] = nc
    return _NC_CACHE["nc"]


def run(x, W1, b1, W2, b2, Wfc, bfc, trace=False, **spmd_kwargs):
    x = np.asarray(x, dtype=np.float32)
    shared = _prep_shared(
        np.asarray(W1, np.float64), np.asarray(b1, np.float32),
        np.asarray(W2, np.float32), np.asarray(b2, np.float32),
        np.asarray(Wfc, np.float32), np.asarray(bfc, np.float32),
    )
    in_maps = []
    for c in range(NCORES):
        xc = x[c * B_LOC : (c + 1) * B_LOC]                    # [128, F, D]
        xtc = np.ascontiguousarray(
            xc.transpose(1, 0, 2).reshape(F, B_LOC * D).astype(np.float16)
        )
        xg16, xg8 = _prep_x(xtc)
        in_maps.append({"xt16": xtc, "xg16": xg16, "xg8": xg8, **shared})
    nc = _get_nc()
    res = run_bass_kernel_spmd(
        nc, in_maps, list(range(NCORES)), trace=trace, **spmd_kwargs
    )
    ys = [np.asarray(res.results[i]["y"]).reshape(B_LOC) for i in range(NCORES)]
    out = np.concatenate(ys).reshape(B, 1).astype(np.float32)
    return out, res


def kernel(x, W1, b1, W2, b2, Wfc, bfc):
    out, _ = run(x, W1, b1, W2, b2, Wfc, bfc, trace=False)
    return out


# revision 46
# speedup vs baseline: 1.6391x; 1.6391x over previous
"""CIN forward kernel for Trainium2, 8 cores.

Symmetric-fold + mixed-dtype-tile pipeline:

Reference computation (per batch b, per position d; n = (b, d) column):
  h0 = x                                  # [F=64, D=64] fields
  for layer l in (1, 2):
    z[(i,j), n] = x[i, n] * h[j, n]       # outer product, K = F*F = 4096
    h[o, n] = relu(sum_c W_l[o, c] z[c, n] + b_l[o])
  y[b] = sum_d concat(h1, h2)[:, b, :] @ wfc + bfc

Distribution: pure data parallel - batch (1024) split 128/core over 8 cores;
weights replicated; no collectives.

Key ideas over the plain 64-chunk pipeline:
  - Layer 1's z1 = x (x) x is SYMMETRIC in (i,j): only 2080 of 4096 products
    are distinct.  We pack the folded products into 17 of the SAME 32 gather
    tiles layer 2 streams (an orientation of K64 with in-degree <= 34 per
    residue assigns every unordered pair {a,b} to a slot (c<17, p) with
    tile_c[p] = x_a and p%64 = b), with folded weights W1f = W1[ab] + W1[ba].
    PE passes drop 64 -> 49 and z-build chunk-ops drop 64 -> 49, with ZERO
    extra DMA (layer 1 free-rides on layer 2's stream).
  - The 10 tiles consumed only by the Pool engine (gpsimd) are streamed in
    fp8-e4m3 (Pool's tensor_tensor cost is dtype-independent; only the x-side
    is quantized, z and W stay fp16 -> final rel err ~9e-3 vs the 2e-2 gate).
    DMA drops 69MB -> ~60MB per core.
  - Tapered 256-col startup blocks and a 256..128 drain tail; fp8 tiles are
    loaded 512-wide once per macro to keep >=512B DMA runs (full DMA rate).
  - W1f stored with duplicated M=128 columns: the idle PE half produces the
    h1 partition-duplicate for free (no cross-partition DMA in the handoff).
  - PE order L1(k+1), L2(k); L2's accumulation runs the 22 DVE-built chunks
    first and the 10 Pool-built chunks LAST, giving the Pool's serial 10us
    z-build the longest possible deadline (this ordering alone is worth 1.6x:
    the reverse order serializes Pool -> L2 -> epi -> Pool into a 27us/block
    limit cycle and collapses the PE to its cold p-state clock).
  - d-pooling as ACT accumulate-copies into a persistent pooled tile; the FC
    collapses to ONE K=128 matmul at the end.  Last block's epilogue runs on
    DVE (idle at drain).
"""

import numpy as np

import concourse.bacc as bacc
import concourse.mybir as mybir
import concourse.tile as tile
from concourse.bass_utils import run_bass_kernel_spmd

F = 64          # fields
D = 64          # embedding dim
B = 1024        # full batch
NCORES = 8
B_LOC = B // NCORES          # 128 batches per core
N_TOTAL = B_LOC * D          # 8192 columns per core
MACRO = 512                  # DRAM blocking width
NCH = 32                     # gather tiles (layer-2 chunks)
L1CH = 17                    # folded layer-1 chunks (tiles 0..16)
NT16 = 22                    # fp16 tiles 0..21 (DVE-consumed)
NT8 = 10                     # fp8 tiles 22..31 (Pool-consumed, layer 2 only)
f16 = mybir.dt.float16
f8 = mybir.dt.float8e4
f32 = mybir.dt.float32

# column blocks (n0, width): big 512 blocks amortize per-op overheads; the
# small 384+128 tail shortens the serial drain chain (the last block's
# z2-build -> L2 -> epilogue -> FC runs at 1/4 width).
BLOCKS = (
    [(i * 256, 256) for i in range(8)]
    + [(2048 + i * 512, 512) for i in range(9)]
    + [(6656 + i * 256, 256) for i in range(5)]
    + [(7936, 128), (8064, 128)]
)
NB = len(BLOCKS)
assert sum(w for _, w in BLOCKS) == N_TOTAL

# DVE piece structure (chunk index ranges into the tile list, aligned with
# the A (0..8) / B (9..16) / C (17..21) DMA pieces)
Z2_DVE_PIECES = [(0, 9), (9, 8), (17, 5)]      # tiles 0..21
Z1_PIECES = [(0, 9), (9, 8)]                   # tiles 0..16
Z2_POOL_PIECES = [(22, 10)]                    # tiles 22..31


def build_cin_nc():
    nc = bacc.Bacc(None)

    xt = nc.dram_tensor("xt16", [F, N_TOTAL], f16, kind="ExternalInput")
    xg16 = nc.dram_tensor(
        "xg16", [(N_TOTAL // MACRO) * 128, NT16 * MACRO], f16,
        kind="ExternalInput",
    )
    xg8 = nc.dram_tensor(
        "xg8", [(N_TOTAL // MACRO) * 128, NT8 * MACRO], f8,
        kind="ExternalInput",
    )
    w1d = nc.dram_tensor("w1fsb", [128, L1CH * 128], f16, kind="ExternalInput")
    w2d = nc.dram_tensor("w2sb", [128, NCH * F], f16, kind="ExternalInput")
    b1d = nc.dram_tensor("b1dup", [128, 1], f32, kind="ExternalInput")
    b2d = nc.dram_tensor("b2dup", [128, 1], f32, kind="ExternalInput")
    wfcd = nc.dram_tensor("wfccat", [128, 1], f32, kind="ExternalInput")
    bfcd = nc.dram_tensor("bfc", [1, 1], f32, kind="ExternalInput")
    yd = nc.dram_tensor("y", [1, B_LOC], f32, kind="ExternalOutput")

    mult = mybir.AluOpType.mult
    Relu = mybir.ActivationFunctionType.Relu
    Copy = mybir.ActivationFunctionType.Copy

    with tile.TileContext(nc) as tc:
        with (
            tc.tile_pool(name="const", bufs=1) as cpool,
            tc.tile_pool(name="xA", bufs=3) as xapool,
            tc.tile_pool(name="xB", bufs=3) as xbpool,
            tc.tile_pool(name="xC", bufs=2) as xcpool,
            tc.tile_pool(name="x8", bufs=3) as x8pool,
            tc.tile_pool(name="xd", bufs=3) as xdpool,
            tc.tile_pool(name="z1", bufs=2) as z1pool,
            tc.tile_pool(name="z2", bufs=2) as z2pool,
            tc.tile_pool(name="zp", bufs=2) as zppool,
            tc.tile_pool(name="h", bufs=3) as hpool,
            tc.tile_pool(name="psum", bufs=4, space="PSUM") as ppool,
            tc.tile_pool(name="psumfc", bufs=1, space="PSUM") as fcpool,
        ):
            w1 = cpool.tile([128, L1CH * 128], f16)
            w2 = cpool.tile([128, NCH * F], f16)
            b1 = cpool.tile([128, 1], f32)
            b2 = cpool.tile([128, 1], f32)
            wfc = cpool.tile([128, 1], f32)
            bfc = cpool.tile([1, 1], f32)
            pooled = cpool.tile([128, B_LOC], f32)   # [0:64]=L1, [64:128]=L2
            y_sb = cpool.tile([1, B_LOC], f32)
            scr = cpool.tile([128, D], f16)          # discard target for accums

            X16 = {}    # k -> dict piece -> tile ('A' tiles 0..8, 'B' 9..16,
                        #                          'C' 17..21)
            X8s = {}    # k -> (tile, col offset); fp8 tiles 22..31, always
                        # loaded as the full 512-wide macro slice
            xds = {}    # k -> x dup tile
            Hps = {}    # k -> h1 dup tile [128, w] fp16
            h2s = {}    # k -> h2 tile (valid on [64:128])
            z1s = {}    # k -> dict c0 -> (tile, c0, ncch)  folded layer-1 z
            z2s = {}    # k -> dict c0 -> (tile, c0, ncch, col_off)
            ps1 = {}
            ps2 = {}

            X_PIECES = {"A": (0, 9), "B": (9, 8), "C": (17, 5)}
            X_POOLS = {"A": xapool, "B": xbpool, "C": xcpool}

            def load_x16(k, piece):
                n0, w = BLOCKS[k]
                c0, ncch = X_PIECES[piece]
                t = X_POOLS[piece].tile([128, ncch * MACRO], f16, tag="X" + piece)
                rb = (n0 // MACRO) * 128
                sub = n0 % MACRO
                src = xg16[rb : rb + 128, :].rearrange(
                    "p (c n) -> p c n", n=MACRO
                )[:, c0 : c0 + ncch, sub : sub + w]
                nc.sync.dma_start(
                    out=t[:, 0 : ncch * w].rearrange("p (c n) -> p c n", n=w),
                    in_=src,
                )
                X16.setdefault(k, {})[piece] = t

            def load_x8(k):
                """fp8 tiles 22..31 for block k.  Always loads the full
                512-wide macro slice (keeps >=512B DMA runs); a 256-block at
                an odd half reuses its even sibling's tile."""
                n0, w = BLOCKS[k]
                if n0 % MACRO != 0:
                    t, _ = X8s[k - 1]
                    X8s[k] = (t, n0 % MACRO)
                    return
                t = x8pool.tile([128, NT8 * MACRO], f8, tag="X8")
                rb = (n0 // MACRO) * 128
                src = xg8[rb : rb + 128, :].rearrange(
                    "p (c n) -> p c n", n=MACRO
                )
                nc.sync.dma_start(
                    out=t[:].rearrange("p (c n) -> p c n", n=MACRO),
                    in_=src,
                )
                X8s[k] = (t, 0)

            def load_xd(k):
                n0, w = BLOCKS[k]
                xd = xdpool.tile([128, MACRO], f16, tag="xd")
                nc.scalar.dma_start(out=xd[0:64, 0:w], in_=xt[:, n0 : n0 + w])
                nc.scalar.dma_start(out=xd[64:128, 0:w], in_=xt[:, n0 : n0 + w])
                xds[k] = xd

            def z1_piece(k, idx, c0=None, ncch=None, eng="dve", tag=None):
                """Folded layer-1 z chunks, piece idx of Z1_PIECES (or an
                explicit chunk range), on DVE or Pool."""
                n0, w = BLOCKS[k]
                if c0 is None:
                    c0, ncch = Z1_PIECES[idx]
                piece, pc0 = ("A", 0) if c0 < 9 else ("B", 9)
                Xt = X16[k][piece]
                # chunk c lives at piece-local index c - pc0
                src = Xt[:, (c0 - pc0) * w : (c0 - pc0 + ncch) * w]
                tag = tag or f"z1{idx}"
                alloc_ch = 4 if tag == "z1p" else Z1_PIECES[idx][1]
                zfull = z1pool.tile([128, alloc_ch * MACRO], f16, tag=tag)
                z = zfull[:, 0 : ncch * w]
                eng_nc = nc.vector if eng == "dve" else nc.gpsimd
                eng_nc.tensor_tensor(
                    z.rearrange("p (f n) -> p f n", n=w),
                    xds[k][:, 0:w].unsqueeze(1).broadcast_to([128, ncch, w]),
                    src.rearrange("p (f n) -> p f n", n=w),
                    mult,
                )
                z1s.setdefault(k, {})[c0] = (zfull, c0, ncch)

            def z1_block(k):
                """All 17 z1 chunks for block k.  Blocks 0-1 put the last 4
                chunks on the (otherwise idle-at-startup) Pool engine."""
                z1_piece(k, 0)
                z1_piece(k, 1)

            def z2_dve_piece(k, idx, split=False):
                n0, w = BLOCKS[k]
                c0, ncch = Z2_DVE_PIECES[idx]
                hdup = Hps[k][:, 0:w]

                def emit(c0_, ncch_):
                    if c0_ < 9:
                        piece, pc0 = "A", 0
                    elif c0_ < 17:
                        piece, pc0 = "B", 9
                    else:
                        piece, pc0 = "C", 17
                    Xt = X16[k][piece]
                    src = Xt[:, (c0_ - pc0) * w : (c0_ - pc0 + ncch_) * w]
                    # per-piece tags: ring of 2 per tag = 2-iteration WAR
                    # slack vs the L2 matmuls that read the previous slab
                    zfull = z2pool.tile(
                        [128, Z2_DVE_PIECES[idx][1] * MACRO], f16,
                        tag=f"z2{idx}"
                    )
                    z = zfull[:, 0 : ncch_ * w]
                    nc.vector.tensor_tensor(
                        z.rearrange("p (f n) -> p f n", n=w),
                        hdup.unsqueeze(1).broadcast_to([128, ncch_, w]),
                        src.rearrange("p (f n) -> p f n", n=w),
                        mult,
                    )
                    z2s.setdefault(k, {})[c0_] = (zfull, c0_, ncch_)

                if not split:
                    emit(c0, ncch)
                else:
                    h1_ = ncch // 2
                    emit(c0, h1_)
                    emit(c0 + h1_, ncch - h1_)

            def z2_pool_piece(k, idx):
                """Layer-2 z chunks on gpsimd from fp8 tiles 22..31."""
                n0, w = BLOCKS[k]
                c0, ncch = Z2_POOL_PIECES[idx]
                hdup = Hps[k][:, 0:w]
                t, coff = X8s[k]
                src = t[:].rearrange(
                    "p (c n) -> p c n", n=MACRO
                )[:, c0 - 22 : c0 - 22 + ncch, coff : coff + w]
                zfull = zppool.tile([128, 10 * MACRO], f16, tag=f"zp{idx}")
                z = zfull[:, 0 : ncch * w]
                nc.gpsimd.tensor_tensor(
                    z.rearrange("p (f n) -> p f n", n=w),
                    hdup.unsqueeze(1).broadcast_to([128, ncch, w]),
                    src,
                    mult,
                )
                z2s.setdefault(k, {})[c0] = (zfull, c0, ncch)

            def pe_layer1(k):
                n0, w = BLOCKS[k]
                ps = ppool.tile([128, MACRO], f32, tag="ps")
                zl = z1s[k]
                for c in range(L1CH):
                    for c0 in zl:
                        zt, zc0, znc = zl[c0]
                        if zc0 <= c < zc0 + znc:
                            off = (c - zc0) * w
                            break
                    nc.tensor.matmul(
                        ps[:, 0:w], w1[:, c * 128 : (c + 1) * 128],
                        zt[:, off : off + w],
                        start=(c == 0), stop=(c == L1CH - 1),
                    )
                ps1[k] = ps
                del z1s[k]

            def pe_layer2(k):
                n0, w = BLOCKS[k]
                ps = ppool.tile([128, MACRO], f32, tag="ps")
                zl = z2s[k]
                # DVE-built chunks first (built early this iteration); the
                # Pool-built chunks (22..31) last — the Pool's 10.3us serial
                # build gets the longest deadline.
                order = list(range(0, 22)) + list(range(22, 32))
                for ci, c in enumerate(order):
                    for c0 in zl:
                        zt, zc0, znc = zl[c0]
                        if zc0 <= c < zc0 + znc:
                            off = (c - zc0) * w
                            break
                    nc.tensor.matmul(
                        ps[64:128, 0:w], w2[:, c * F : (c + 1) * F],
                        zt[:, off : off + w],
                        start=(ci == 0), stop=(ci == NCH - 1),
                    )
                ps2[k] = ps
                del z2s[k]

            def epi1(k):
                n0, w = BLOCKS[k]
                Hp = hpool.tile([128, MACRO], f16, tag="Hp")
                # epi1 gates the Pool's z2 build: schedule it ahead of the
                # (slack-rich) pooling reductions in the ACT queue
                with tc.high_priority():
                    nc.scalar.activation(
                        Hp[:, 0:w], ps1[k][:, 0:w], Relu, bias=b1[:, :]
                    )
                Hps[k] = Hp
                del ps1[k]

            def epi2(k):
                n0, w = BLOCKS[k]
                h2 = hpool.tile([128, MACRO], f16, tag="h2")
                if k == NB - 1:
                    # tail: bias+relu on DVE (idle) to skip the ACT hop
                    nc.vector.tensor_scalar(
                        h2[64:128, 0:w], ps2[k][64:128, 0:w], b2[64:128, :],
                        0.0, mybir.AluOpType.add, mybir.AluOpType.max,
                    )
                else:
                    nc.scalar.activation(
                        h2[64:128, 0:w], ps2[k][64:128, 0:w], Relu,
                        bias=b2[64:128, :],
                    )
                h2s[k] = h2
                del ps2[k]

            def red1(k):
                n0, w = BLOCKS[k]
                nb = w // D
                cb = n0 // D
                for j in range(nb):
                    nc.scalar.activation(
                        scr[0:64, :], Hps[k][0:64, j * D : (j + 1) * D], Copy,
                        accum_out=pooled[0:64, cb + j : cb + j + 1],
                    )

            def red2(k):
                n0, w = BLOCKS[k]
                nb = w // D
                cb = n0 // D
                if k == NB - 1:
                    # tail: ACT accums would delay the final FC; DVE is idle
                    nc.vector.tensor_reduce(
                        pooled[64:128, cb : cb + nb],
                        h2s[k][64:128, 0:w].rearrange("p (b d) -> p b d", d=D),
                        mybir.AxisListType.X, mybir.AluOpType.add,
                    )
                else:
                    for j in range(nb):
                        nc.scalar.activation(
                            scr[64:128, :],
                            h2s[k][64:128, j * D : (j + 1) * D], Copy,
                            accum_out=pooled[64:128, cb + j : cb + j + 1],
                        )
                del h2s[k]

            def load_block(k):
                load_xd(k)
                load_x16(k, "A")
                load_x8(k)
                load_x16(k, "B")


            # ---------------- prologue ----------------
            # blocks 0,1: tiles + z1 built immediately; weights interleaved
            load_block(0)
            load_x16(0, "C")
            nc.scalar.dma_start(out=w1[:], in_=w1d[:])
            z1_block(0)
            load_block(1)
            nc.scalar.dma_start(out=b1[:], in_=b1d[:])
            nc.scalar.dma_start(out=w2[:], in_=w2d[:])
            nc.scalar.dma_start(out=b2[:], in_=b2d[:])
            nc.scalar.dma_start(out=wfc[:], in_=wfcd[:])
            nc.scalar.dma_start(out=bfc[:], in_=bfcd[:])
            z1_block(1)
            pe_layer1(0)
            epi1(0)
            # Pool starts the very first l2 chunks for block 0
            z2_pool_piece(0, 0)

            # ---------------- steady state ----------------
            # iteration k: PE runs L1(k+1), L2(k).
            #   DVE: z2(k) x3, then z1(k+2) x2 (tiles for k+2 land mid-iter).
            #   Pool: z2_pool(k+1) after epi1(k+1).
            #   DMA: block k+2 piece A/x8/B/xd, then C(k+1).
            for k in range(NB):
                if k + 1 < NB:
                    pe_layer1(k + 1)
                    epi1(k + 1)
                    z2_pool_piece(k + 1, 0)
                z2_dve_piece(k, 0)
                if k + 2 < NB:
                    load_block(k + 2)
                z2_dve_piece(k, 1)
                if k + 1 < NB:
                    load_x16(k + 1, "C")
                z2_dve_piece(k, 2, split=(k == NB - 1))
                if k + 2 < NB:
                    z1_block(k + 2)
                pe_layer2(k)
                epi2(k)
                if k + 1 < NB:
                    red1(k + 1)
                if k == 0:
                    red1(0)
                red2(k)

            # ---------------- FC ----------------
            yp = fcpool.tile([1, B_LOC], f32)
            nc.tensor.matmul(yp[:], wfc[:], pooled[:], start=True, stop=True)
            nc.vector.tensor_scalar_add(y_sb[:], yp[:], bfc[:])
            nc.sync.dma_start(out=yd[:], in_=y_sb[:])

    return nc


def _build_fold():
    """Orientation of K64 (+self-loops) assigning every unordered pair {a,b}
    to a slot (c < 17, p) with in1 = x[f[c,p]] and in0 = x[p % 64], while the
    full 32-tile family covers every ORDERED pair exactly once for layer 2.

    Returns f [32, 128] int (the gather patterns)."""
    L1 = [[] for _ in range(F)]
    for a in range(F):
        L1[a].append(a)                      # self loop (a,a) -> residue a
    for a in range(F):
        for b in range(a + 1, F):
            d = (b - a) % F
            if d == 32:
                r, v = a, b                  # tie: to the smaller endpoint
            elif 1 <= d <= 31:
                r, v = b, a
            else:
                r, v = a, b
            L1[r].append(v)
    seqs = np.zeros((F, F), dtype=np.int64)
    for j in range(F):
        s = list(L1[j])
        assert len(s) <= 2 * L1CH
        used = set(s)
        pads = [v for v in range(F) if v not in used]
        while len(s) < 2 * L1CH:
            s.append(pads.pop())
        rest = [v for v in range(F) if v not in set(s)]
        seqs[j] = s + rest
        assert sorted(seqs[j].tolist()) == list(range(F))
    f = np.zeros((NCH, 128), dtype=np.int64)
    for c in range(NCH):
        f[c, 0:64] = seqs[:, 2 * c]
        f[c, 64:128] = seqs[:, 2 * c + 1]
    return f


_FOLD = _build_fold()


def _prep_shared(W1, b1, W2, b2, Wfc, bfc):
    """Host-side weight relayout (replicated on every core)."""
    f = _FOLD

    # layer-1 folded weights, M = 128 duplicated columns
    Wf = np.zeros((F, L1CH, 128), dtype=np.float64)
    assigned = set()
    for c in range(L1CH):
        for p in range(128):
            a = int(f[c, p])
            b = p % 64
            key = (min(a, b), max(a, b))
            if key in assigned:
                continue
            assigned.add(key)
            if a == b:
                Wf[:, c, p] = W1[:, a * F + a]
            else:
                Wf[:, c, p] = W1[:, a * F + b] + W1[:, b * F + a]
    assert len(assigned) == F * (F + 1) // 2
    # w1fsb[p, c*128 + m] = Wf[m % 64, c, p]
    w1fsb = np.ascontiguousarray(
        np.concatenate([Wf, Wf], axis=0)          # [128 m, 17, 128 p]
        .transpose(2, 1, 0).reshape(128, L1CH * 128)
    ).astype(np.float16)

    # layer-2 permuted weights: w2sb[p, c*64 + o] = W2[o, f[c,p]*64 + p%64]
    cols = f * F + (np.arange(128) % 64)[None, :]     # [32, 128]
    w2p = W2[:, cols.reshape(-1)].reshape(F, NCH, 128)  # [o, c, p]
    w2sb = np.ascontiguousarray(
        w2p.transpose(2, 1, 0).reshape(128, NCH * F)
    ).astype(np.float16)

    return {
        "w1fsb": w1fsb,
        "w2sb": w2sb,
        "b1dup": np.concatenate([b1, b1]).reshape(128, 1).astype(np.float32),
        "b2dup": np.concatenate([b2, b2]).reshape(128, 1).astype(np.float32),
        "wfccat": Wfc.reshape(128, 1).astype(np.float32),
        "bfc": bfc.reshape(1, 1).astype(np.float32),
    }


def _prep_x(xtc):
    """Per-macro-blocked gather tile layouts.
    xg16[m*128+p, k*MACRO+nn] = xtc[f[k,p],  m*MACRO+nn]  (tiles 0..21, fp16)
    xg8 [m*128+p, k*MACRO+nn] = xtc[f[22+k,p], m*MACRO+nn] (tiles 22..31, fp8)
    """
    import ml_dtypes

    f = _FOLD
    nm = N_TOTAL // MACRO
    gath = xtc[f.reshape(-1)].reshape(NCH, 128, nm, MACRO)   # [c, p, m, nn]
    g = np.ascontiguousarray(gath.transpose(2, 1, 0, 3))     # [m, p, c, nn]
    xg16 = np.ascontiguousarray(
        g[:, :, 0:NT16, :].reshape(nm * 128, NT16 * MACRO)
    )
    xg8 = np.ascontiguousarray(
        g[:, :, NT16:NCH, :].astype(np.float32)
        .reshape(nm * 128, NT8 * MACRO)
    ).astype(ml_dtypes.float8_e4m3)
    return xg16, xg8


_NC_CACHE = {}


def _get_nc():
    if "nc" not in _NC_CACHE:
        nc = build_cin_nc()
        nc.finalize()
        _NC_CACHE["nc"] = nc
    return _NC_CACHE["nc"]


def run(x, W1, b1, W2, b2, Wfc, bfc, trace=False, **spmd_kwargs):
    x = np.asarray(x, dtype=np.float32)
    shared = _prep_shared(
        np.asarray(W1, np.float64), np.asarray(b1, np.float32),
        np.asarray(W2, np.float32), np.asarray(b2, np.float32),
        np.asarray(Wfc, np.float32), np.asarray(bfc, np.float32),
    )
    in_maps = []
    for c in range(NCORES):
        xc = x[c * B_LOC : (c + 1) * B_LOC]                    # [128, F, D]
        xtc = np.ascontiguousarray(
            xc.transpose(1, 0, 2).reshape(F, B_LOC * D).astype(np.float16)
        )
        xg16, xg8 = _prep_x(xtc)
        in_maps.append({"xt16": xtc, "xg16": xg16, "xg8": xg8, **shared})
    nc = _get_nc()
    res = run_bass_kernel_spmd(
        nc, in_maps, list(range(NCORES)), trace=trace, **spmd_kwargs
    )
    ys = [np.asarray(res.results[i]["y"]).reshape(B_LOC) for i in range(NCORES)]
    out = np.concatenate(ys).reshape(B, 1).astype(np.float32)
    return out, res


def kernel(x, W1, b1, W2, b2, Wfc, bfc):
    out, _ = run(x, W1, b1, W2, b2, Wfc, bfc, trace=False)
    return out
